# revision 33
# baseline (speedup 1.0000x reference)
"""CapsNet-BCL Trainium2 kernel: 8-core SPMD Bass/Tile implementation.

Host algebra: fc1/fc2 have no nonlinearity between them, so
Weff[t] = fc2_w[t] @ fc1_w[t], beff[t] = fc2_w[t]@fc1_b[t]+fc2_b[t] and
h2 = x @ Weff[t].T + beff[t].  Only tasks r <= eval_t route (softmax mask
-10000 underflows to exactly 0 in fp32), so only route_weights[:, :eval_t+1]
is read.

Sharding: core k computes h2/sem for batches [8k, 8k+8); sem is AllGathered
in two task chunks ({r0..3} then {r4..}); core c computes priors+routing for
capsule c over all 64 batches.  The torch flat view vote(CAP,B,1,L)->
(B,L,CAP) maps output batch b to vote capsule b//8, so core c's vote is
exactly what output batches [8c,8c+8) need: each core emits its own output
slice, no second collective.

Numerics: the routing softmax saturates (|logits| to ~200, top-2 gaps down
to ~2.5), so priors need ~1e-4 relative accuracy — everything in the priors
path stays f32/f32r.

Perf structure vs the original baseline:
 - phase 1 stays in the matmul's natural [(t,c), token] layout: the squash
   norm over t is a 0/1-selector matmul, the per-(c,token) scale is
   replicated back over t with a second tiny matmul, and sem is written to
   DRAM with contiguous 2KB runs (48 descriptors/write instead of 768 —
   HWDGE descriptor generation was the old phase-1 pacing bottleneck).
 - x loaded token-chunk-major, pipelined with the phase-1 matmuls.
 - rw prefetched right after phase 1 (explicit dep) so x gets the full
   HBM pipe first and rw streams during the AllGather window.
 - a tiny warm-up AllGather at t=0 absorbs the first-collective setup.
 - final-linear bias folded into the matmul as a 9th contraction row.
"""

import sys

import numpy as np

if "/opt/trn_rl_repo" not in sys.path:
    sys.path.insert(0, "/opt/trn_rl_repo")

NTASKS = 10
CAP = 8
L = 256
D = 768
B = 64
N_CORES = 8
BL = B // N_CORES          # batches per core (8)
TOK = BL * L               # tokens per core (2048)
KT = D // 128              # k tiles over D (6)
IT = (L * CAP) // 128      # i tiles over L*CAP (16)
NT = TOK // 512            # phase-1 moving chunks (4)

_CACHE = {}


def _build(A, use_cc=True):
    """Build the 8-core SPMD Bass program for A = eval_t+1 active tasks."""
    import concourse.bass as bass
    import concourse.tile as tile
    import concourse.mybir as mybir
    from concourse import bacc
    from concourse.tile import add_dep_helper

    f32 = mybir.dt.float32
    f32r = mybir.dt.float32r
    Alu = mybir.AluOpType
    Act = mybir.ActivationFunctionType
    X = mybir.AxisListType.X

    nc = bacc.Bacc("TRN2", target_bir_lowering=False, debug=False,
                   num_devices=N_CORES)

    TC = NTASKS * CAP  # 80
    AC = A * CAP
    NPAIR = (A + 1) // 2   # task-pair transpose tiles
    # a single AllGather: collective latency here is dominated by a ~9us
    # ncfw polling cadence per algorithm step, so one op beats any chunking
    RLO = A
    CH = [A]
    CH0 = [0]

    xT = nc.dram_tensor("xT", [D, TOK], f32r, kind="ExternalInput").ap()
    weffT = nc.dram_tensor("weffT", [D, TC], f32r, kind="ExternalInput").ap()
    beff_col = nc.dram_tensor("beff_col", [TC, 1], f32,
                              kind="ExternalInput").ap()
    # rw_h[p, (r, k, o)] = route_weights[core, r, i2(k,p), o] where the
    # contraction index is reordered to i2 = c*L + l (phase-1 sem rows are
    # (t, c) with token cols, so gathered sem transposes to (c, l) order)
    rw = nc.dram_tensor("rw", [128, A * IT * L], f32r,
                        kind="ExternalInput").ap()
    # wlT9 = [larger_w[e].T; larger_b[e]] -- bias folded in as a 9th
    # contraction row so phase 6 needs no separate bias add
    wlT9 = nc.dram_tensor("wlT9", [CAP + 1, D], f32r,
                          kind="ExternalInput").ap()
    ones_row = nc.dram_tensor("ones_row", [1, 32 * B], f32,
                              kind="ExternalInput").ap()
    # squash helpers: selT[(t,c), c'] = (c == c'); repT[c, (t<A,c')] = (c==c')
    selT = nc.dram_tensor("selT", [TC, CAP], f32r,
                          kind="ExternalInput").ap()
    repT = nc.dram_tensor("repT", [CAP, AC], f32r,
                          kind="ExternalInput").ap()
    ident = nc.dram_tensor("ident", [128, 128], f32, kind="ExternalInput").ap()
    out = nc.dram_tensor("out", [BL, L, D], f32, kind="ExternalOutput").ap()

    # collective chunks by task: rows (t, c) t-major, cols (b_l, l)
    sem_p = [nc.dram_tensor(f"sem_p{i}", [n * CAP, TOK], f32).ap()
             for i, n in enumerate(CH)]
    gath_p = [nc.dram_tensor(f"gath_p{i}", [N_CORES * n * CAP, TOK], f32,
                             addr_space="Shared").ap()
              for i, n in enumerate(CH)]
    # tiny collective to absorb the first-op ncfw/channel setup cost
    # while phase 1 is still computing
    cc_warm_in = nc.dram_tensor("cc_warm_in", [1, 16], f32).ap()
    cc_warm_out = nc.dram_tensor("cc_warm_out", [N_CORES, 16], f32,
                                 addr_space="Shared").ap()
    voteT_dram = nc.dram_tensor("voteT_dram", [L, B], f32).ap()

    with tile.TileContext(nc) as tc:
        with tc.tile_pool(name="singles", bufs=1) as singles:
            # ---- constants ----
            weff_sb = singles.tile([128, KT * TC], f32r)
            nc.sync.dma_start(out=weff_sb,
                              in_=weffT.rearrange("(k p) c -> p k c", p=128))
            beff_sb = singles.tile([TC, 1], f32)
            nc.sync.dma_start(out=beff_sb, in_=beff_col)
            ident_sb = singles.tile([128, 128], f32)
            nc.sync.dma_start(out=ident_sb, in_=ident)
            wlT_sb = singles.tile([CAP + 1, D], f32r)
            nc.sync.dma_start(out=wlT_sb, in_=wlT9)
            sel_sb = singles.tile([TC, CAP], f32r)
            nc.sync.dma_start(out=sel_sb, in_=selT)
            rep_sb = singles.tile([CAP, AC], f32r)
            nc.sync.dma_start(out=rep_sb, in_=repT)

            priors_sb = singles.tile([64, A * L], f32)
            semT_sb = singles.tile([128, NPAIR * IT * 128], f32r)

            rw_sb = []
            for r in range(A):
                rwt = singles.tile([128, IT * L], f32r, tag=f"rw{r}")
                rw_sb.append(rwt)

            # ===== Phase 1: semantic stage, batch-parallel ================
            # All in the [(t,c), token] layout h2 is produced in:
            #   h2a[80, 512] (+bias via ACT); h2sq = h2a^2 (ACT);
            #   sq[c, tok] = selT.T @ h2sq (PE); scal = sqrt(sq)/(1+sq)
            #   (DVE/ACT on [8, 512]); scal_rep = repT.T @ scal (PE);
            #   sem = h2a[:AC] * scal_rep (DVE) -> contiguous DRAM write.
            with (
                tc.tile_pool(name="x_pool", bufs=12) as xpool,
                tc.tile_pool(name="pA", bufs=2, space="PSUM") as pA,
                tc.tile_pool(name="pS", bufs=2, space="PSUM") as pS,
                tc.tile_pool(name="pR", bufs=2, space="PSUM") as pR,
                tc.tile_pool(name="h2a_pool", bufs=2) as hapool,
                tc.tile_pool(name="sem_pool", bufs=3) as spool,
                tc.tile_pool(name="sq_pool", bufs=3) as qpool,
            ):
                last_sem_write = None
                for nt in range(NT):            # 4 chunks of 512 tokens
                    xks = []
                    for k in range(KT):
                        xk = xpool.tile([128, 512], f32r, tag="xk")
                        nc.sync.dma_start(
                            out=xk,
                            in_=xT[k * 128:(k + 1) * 128,
                                   nt * 512:(nt + 1) * 512])
                        xks.append(xk)
                    psa = pA.tile([TC, 512], f32, tag="psa")
                    for k in range(KT):
                        nc.tensor.matmul(
                            psa,
                            lhsT=weff_sb[:, k * TC:(k + 1) * TC],
                            rhs=xks[k],
                            start=(k == 0), stop=(k == KT - 1),
                        )
                    h2a = hapool.tile([TC, 512], f32, tag="h2a")
                    nc.scalar.activation(h2a, psa, Act.Identity,
                                         bias=beff_sb)
                    h2sq = spool.tile([TC, 512], f32r, tag="h2sq")
                    nc.scalar.activation(h2sq, h2a, Act.Square)
                    psq = pS.tile([CAP, 512], f32, tag="psq")
                    nc.tensor.matmul(psq, lhsT=sel_sb, rhs=h2sq,
                                     start=True, stop=True)
                    # scal = sqrt(sq)/(1+sq); the reciprocal goes through
                    # ACT Ln+Exp -- DVE's iterative reciprocal costs 8
                    # passes x 512 lanes-elements here (~4.3us per chunk)
                    rt = qpool.tile([CAP, 512], f32, tag="rt")
                    nc.scalar.activation(rt, psq, Act.Sqrt)
                    den = qpool.tile([CAP, 512], f32, tag="den")
                    nc.vector.tensor_scalar_add(den, psq, 1.0)
                    lnd = qpool.tile([CAP, 512], f32, tag="lnd")
                    nc.scalar.activation(lnd, den, Act.Ln)
                    rden = qpool.tile([CAP, 512], f32, tag="rden")
                    nc.scalar.activation(rden, lnd, Act.Exp, scale=-1.0)
                    scal = qpool.tile([CAP, 512], f32r, tag="scal")
                    nc.vector.tensor_mul(scal, rt, rden)
                    prep = pR.tile([AC, 512], f32, tag="prep")
                    nc.tensor.matmul(prep, lhsT=rep_sb, rhs=scal,
                                     start=True, stop=True)
                    sem = spool.tile([AC, 512], f32, tag="sem")
                    nc.vector.tensor_tensor(out=sem, in0=h2a[:AC],
                                            in1=prep, op=Alu.mult)
                    for i, n in enumerate(CH):
                        wr = nc.sync.dma_start(
                            out=sem_p[i][:, nt * 512:(nt + 1) * 512],
                            in_=sem[CH0[i] * CAP:(CH0[i] + n) * CAP])
                        last_sem_write = wr

            # ---- rw prefetch: gated behind phase 1 so the x loads get the
            # full HBM pipe first; rw then streams during the AllGathers ----
            for r in range(A):
                ld = nc.scalar.dma_start(
                    out=rw_sb[r], in_=rw[:, r * IT * L:(r + 1) * IT * L])
                add_dep_helper(last_sem_write.ins, ld.ins, sync=True,
                               reason="rw prefetch after phase-1 traffic")

            # ---- PE keep-warm through the AllGather window: ~20us of
            # back-to-back dummy matmuls so phase 3/4 runs at 2.4 GHz ----
            with tc.tile_pool(name="pW", bufs=2, space="PSUM") as pW:
                for w in range(80):
                    pdw = pW.tile([TC, 512], f32, tag="pdw")
                    mm = nc.tensor.matmul(
                        pdw[:, 0:448], lhsT=weff_sb[:, 0:TC],
                        rhs=weff_sb[:, 0:448],
                        start=True, stop=True)
                    if w == 0:
                        add_dep_helper(last_sem_write.ins, mm.ins,
                                       sync=True,
                                       reason="warm PE during AllGather")

            # ===== Phase 2: allgather sem (task chunks) ===================
            if use_cc:
                for i in range(len(CH)):
                    nc.gpsimd.collective_compute(
                        "AllGather", Alu.bypass,
                        replica_groups=[list(range(N_CORES))],
                        ins=[sem_p[i][:]], outs=[gath_p[i][:]])
            else:
                for i in range(len(CH)):
                    nc.sync.dma_start(out=gath_p[i][0:CH[i] * CAP],
                                      in_=sem_p[i][:])

            # ===== Phase 3+4: gather-transpose + priors ===================
            # Pair tiles: tile t holds tasks (2t, 2t+1) x all 64 batches,
            # partition = (task, rank, b_l), cols = (c, l); PE transposes
            # give semT[(c,l)-slice, (task, batch)] for the priors lhsT.
            NP = NPAIR

            def chunk_r(r):
                return (0, r) if r < RLO else (1, r - RLO)

            with (
                tc.tile_pool(name="gpool", bufs=2) as gpool,
                tc.tile_pool(name="pT", bufs=4, space="PSUM") as pT,
                tc.tile_pool(name="pP", bufs=3, space="PSUM") as pP,
            ):
                g_tiles = []
                for t in range(NP):
                    g_sb = gpool.tile([128, L * CAP], f32, tag="g")
                    for ri in range(2):
                        if 2 * t + ri >= A:
                            continue
                        ci, rloc = chunk_r(2 * t + ri)
                        for rank in range(N_CORES):
                            base = (rank * CH[ci] + rloc) * CAP
                            eng = nc.sync if (rank % 2 == 0) else nc.scalar
                            eng.dma_start(
                                out=g_sb[ri * 64 + rank * 8:
                                         ri * 64 + rank * 8 + 8].rearrange(
                                    "p (c l) -> p c l", c=CAP),
                                in_=gath_p[ci][base:base + CAP, :].rearrange(
                                    "c (b l) -> b c l", b=BL))
                    g_tiles.append(g_sb)

                for t in range(NP):
                    for k in range(IT):
                        psT = pT.tile([128, 128], f32, tag="psT")
                        nc.tensor.transpose(
                            psT, in_=g_tiles[t][:, k * 128:(k + 1) * 128],
                            identity=ident_sb)
                        cp = nc.vector if (k % 3) else nc.scalar
                        dst = semT_sb[:, (t * IT + k) * 128:
                                      (t * IT + k + 1) * 128]
                        if cp is nc.vector:
                            nc.vector.tensor_copy(out=dst, in_=psT)
                        else:
                            nc.scalar.activation(dst, psT, Act.Copy)
                    for ri in range(2):
                        r = 2 * t + ri
                        if r >= A:
                            continue
                        pp = pP.tile([64, L], f32, tag="pp")
                        for k in range(IT):
                            base = (t * IT + k) * 128 + ri * 64
                            nc.tensor.matmul(
                                pp, lhsT=semT_sb[:, base:base + 64],
                                rhs=rw_sb[r][:, k * L:(k + 1) * L],
                                start=(k == 0), stop=(k == IT - 1))
                        cp = nc.vector if (r % 2 == 0) else nc.scalar
                        dst = priors_sb[:, r * L:(r + 1) * L]
                        if cp is nc.vector:
                            nc.vector.tensor_copy(out=dst, in_=pp)
                        else:
                            nc.scalar.activation(dst, pp, Act.Copy)

            # ===== Phase 5: routing (vectorized over r) ===================
            with (
                tc.tile_pool(name="route", bufs=1) as rp,
                tc.tile_pool(name="pV", bufs=2, space="PSUM") as pV,
            ):
                vote = rp.tile([64, L], f32)
                scr = rp.tile([64, L], f32)
                big = rp.tile([64, A * L], f32)
                l1 = rp.tile([64, A], f32)
                l2 = rp.tile([64, A], f32)
                dots_raw = rp.tile([64, A], f32)
                dots = rp.tile([64, A], f32)
                ex = rp.tile([64, A], f32)
                probs = rp.tile([64, A], f32)
                n2 = rp.tile([64, 1], f32)
                rt2 = rp.tile([64, 1], f32)
                den2 = rp.tile([64, 1], f32)
                rden2 = rp.tile([64, 1], f32)
                sc2 = rp.tile([64, 1], f32)
                mx = rp.tile([64, 1], f32)
                nmx = rp.tile([64, 1], f32)
                ssum = rp.tile([64, 1], f32)
                rsum = rp.tile([64, 1], f32)

                def warm(dep, m):
                    # tiny matmul with a true dep on the routing chain --
                    # keeps the PE HAM un-throttled through phase 5
                    pdum = pV.tile([64, 128], f32, tag="pdum")
                    nc.tensor.matmul(pdum[:m], lhsT=dep[:, 0:m],
                                     rhs=priors_sb[:, 0:128],
                                     start=True, stop=True)

                def squash_scal():
                    # sc2 = sqrt(n2)/(1+n2); outsq = sc2*vote is never
                    # materialized -- dots get scaled by sc2 instead.
                    nc.vector.tensor_mul(scr, vote, vote)
                    nc.vector.tensor_reduce(out=n2, in_=scr, axis=X,
                                            op=Alu.add)
                    nc.scalar.activation(rt2, n2, Act.Sqrt)
                    nc.vector.tensor_scalar_add(den2, n2, 1.0)
                    nc.vector.reciprocal(rden2, den2)
                    nc.vector.tensor_mul(sc2, rt2, rden2)

                def logit_update(l_prev, l_new):
                    for r in range(A):
                        nc.vector.scalar_tensor_tensor(
                            out=big[:, r * L:(r + 1) * L],
                            in0=priors_sb[:, r * L:(r + 1) * L],
                            scalar=1.0, in1=vote,
                            op0=Alu.mult, op1=Alu.mult,
                            accum_out=dots_raw[:, r:r + 1])
                    if l_prev is None:
                        nc.vector.tensor_scalar_mul(l_new, dots_raw, sc2)
                    else:
                        nc.vector.tensor_scalar_mul(dots, dots_raw, sc2)
                        nc.vector.tensor_add(l_new, dots, l_prev)

                def softmax_vote(l_in):
                    nc.vector.tensor_reduce(out=mx, in_=l_in, axis=X,
                                            op=Alu.max)
                    nc.vector.tensor_scalar_mul(nmx, mx, -1.0)
                    nc.scalar.activation(ex, l_in, Act.Exp, bias=nmx,
                                         accum_out=ssum)
                    nc.vector.reciprocal(rsum, ssum)
                    nc.vector.tensor_scalar_mul(probs, ex, rsum)
                    pr_b = bass.AP(
                        tensor=probs.tensor, offset=probs.offset,
                        ap=[probs.ap[0], [1, A], [0, L]])
                    nc.vector.tensor_tensor(
                        out=big.rearrange("p (r o) -> p r o", r=A),
                        in0=priors_sb.rearrange("p (r o) -> p r o", r=A),
                        in1=pr_b, op=Alu.mult)
                    nc.vector.tensor_reduce(
                        out=vote,
                        in_=big.rearrange("p (r o) -> p o r", r=A),
                        axis=X, op=Alu.add)

                # iter 1: uniform probs = 1/A
                nc.vector.tensor_reduce(
                    out=scr,
                    in_=priors_sb.rearrange("p (r o) -> p o r", r=A),
                    axis=X, op=Alu.add)
                nc.vector.tensor_scalar_mul(vote, scr, 1.0 / A)
                squash_scal()
                warm(vote, 64)
                logit_update(None, l1)
                warm(l1, A)
                softmax_vote(l1)
                warm(vote, 64)
                squash_scal()
                logit_update(l1, l2)
                warm(l2, A)
                softmax_vote(l2)

                # transpose vote [64, 256] -> voteT_dram [256, 64]
                vT_sb = rp.tile([128, 128], f32)
                for half in range(2):
                    pv = pV.tile([128, 64], f32, tag="pv")
                    nc.tensor.transpose(
                        pv, in_=vote[:, half * 128:(half + 1) * 128],
                        identity=ident_sb[:64, :64])
                    nc.vector.tensor_copy(
                        out=vT_sb[:, half * 64:(half + 1) * 64], in_=pv)
                    nc.sync.dma_start(
                        out=voteT_dram[half * 128:(half + 1) * 128],
                        in_=vT_sb[:, half * 64:(half + 1) * 64])

            # ===== Phase 6: final linear ==================================
            # voteT_dram[o, b]; h_blT[cap, l] = voteT[(l%32)*8+cap,
            # b_l*8 + l//32].  vt2[cap, (lr, b)] loads with 256B bursts;
            # row CAP is all-ones so wlT9's bias row lands in the matmul.
            with (
                tc.tile_pool(name="vt", bufs=1) as vtp,
                tc.tile_pool(name="pF", bufs=4, space="PSUM") as pF,
                tc.tile_pool(name="outp", bufs=3) as op_,
            ):
                vt2 = vtp.tile([CAP + 1, 32 * B], f32)
                src = bass.AP(
                    tensor=voteT_dram.tensor, offset=voteT_dram.offset,
                    ap=[[B, CAP], [CAP * B, 32], [1, B]])
                nc.sync.dma_start(out=vt2[:CAP], in_=src)
                nc.sync.dma_start(out=vt2[CAP:CAP + 1], in_=ones_row)
                # permute free layout (lr, b) -> (b, lr) during the f32r
                # convert, so each lhsT is a contiguous 128-col slice
                vt2r = vtp.tile([CAP + 1, 32 * B], f32r)
                nc.vector.tensor_copy(
                    out=vt2r.rearrange("p (b lr) -> p b lr", lr=32),
                    in_=vt2.rearrange("p (lr b) -> p b lr", lr=32))
                NH = 2
                for b_l in range(BL):
                    for lt in range(2):
                        o_sb = op_.tile([128, D], f32, tag="o")
                        lhsT = vt2r[:, (b_l * CAP + lt * 4) * 32:
                                    (b_l * CAP + lt * 4) * 32 + 128]
                        for nh in range(NH):
                            pf = pF.tile([128, D // NH], f32, tag="pf")
                            nc.tensor.matmul(
                                pf, lhsT=lhsT,
                                rhs=wlT_sb[:, nh * (D // NH):
                                           (nh + 1) * (D // NH)],
                                start=True, stop=True)
                            dst = o_sb[:, nh * (D // NH):(nh + 1) * (D // NH)]
                            if nh == 0:
                                nc.vector.tensor_copy(out=dst, in_=pf)
                            else:
                                nc.scalar.activation(dst, pf, Act.Copy)
                        nc.sync.dma_start(
                            out=out[b_l, lt * 128:(lt + 1) * 128, :],
                            in_=o_sb)

    nc.compile()
    return nc


def _host_prep(x, fc1_w, fc1_b, fc2_w, fc2_b, route_weights, larger_w,
               larger_b, eval_t):
    A = int(eval_t) + 1
    f64 = np.float64
    weff = np.einsum("tcd,tdi->tci", fc2_w.astype(f64), fc1_w.astype(f64))
    beff = (np.einsum("tcd,td->tc", fc2_w.astype(f64), fc1_b.astype(f64))
            + fc2_b.astype(f64))
    weffT = np.ascontiguousarray(
        weff.reshape(NTASKS * CAP, D).T).astype(np.float32)
    beff_col = beff.reshape(NTASKS * CAP, 1).astype(np.float32)
    wlT9 = np.ascontiguousarray(np.concatenate(
        [larger_w[int(eval_t)].T, larger_b[int(eval_t)].reshape(1, D)],
        axis=0)).astype(np.float32)
    ones_row = np.ones((1, 32 * B), dtype=np.float32)
    selT = np.tile(np.eye(CAP, dtype=np.float32), (NTASKS, 1))
    repT = np.tile(np.eye(CAP, dtype=np.float32), (1, A))
    ident = np.eye(128, dtype=np.float32)

    in_maps = []
    for c in range(N_CORES):
        xT_c = np.ascontiguousarray(
            x[c * BL:(c + 1) * BL].reshape(TOK, D).T).astype(np.float32)
        # reorder the contraction index to i2 = c2*L + l, then k-tile:
        # rw_c[p, (r, k, o)] = route_weights[c, r, l(k,p)*CAP + c2(k,p), o]
        rw2 = route_weights[c, :A].reshape(A, L, CAP, L).transpose(0, 2, 1, 3)
        rw_c = np.ascontiguousarray(
            rw2.reshape(A, IT, 128, L)
            .transpose(2, 0, 1, 3).reshape(128, A * IT * L)).astype(
                np.float32)
        in_maps.append({
            "xT": xT_c, "weffT": weffT, "beff_col": beff_col, "rw": rw_c,
            "wlT9": wlT9, "ones_row": ones_row, "selT": selT, "repT": repT,
            "ident": ident,
        })
    return A, in_maps


def kernel(**inputs):
    from concourse.bass_utils import run_bass_kernel_spmd

    A, in_maps = _host_prep(**inputs)
    if A not in _CACHE:
        _CACHE[A] = _build(A)
    nc = _CACHE[A]
    res = run_bass_kernel_spmd(nc, in_maps, core_ids=list(range(N_CORES)))
    return np.concatenate(
        [res.results[c]["out"] for c in range(N_CORES)], axis=0)


# revision 36
# speedup vs baseline: 1.0277x; 1.0277x over previous
"""CapsNet-BCL Trainium2 kernel: 8-core SPMD Bass/Tile implementation.

Host algebra: fc1/fc2 have no nonlinearity between them, so
Weff[t] = fc2_w[t] @ fc1_w[t], beff[t] = fc2_w[t]@fc1_b[t]+fc2_b[t] and
h2 = x @ Weff[t].T + beff[t].  Only tasks r <= eval_t route (softmax mask
-10000 underflows to exactly 0 in fp32), so only route_weights[:, :eval_t+1]
is read.

Sharding: core k computes h2/sem for batches [8k, 8k+8); sem is AllGathered
in two task chunks ({r0..3} then {r4..}); core c computes priors+routing for
capsule c over all 64 batches.  The torch flat view vote(CAP,B,1,L)->
(B,L,CAP) maps output batch b to vote capsule b//8, so core c's vote is
exactly what output batches [8c,8c+8) need: each core emits its own output
slice, no second collective.

Numerics: the routing softmax saturates (|logits| to ~200, top-2 gaps down
to ~2.5), so priors need ~1e-4 relative accuracy — everything in the priors
path stays f32/f32r.

Perf structure vs the original baseline:
 - phase 1 stays in the matmul's natural [(t,c), token] layout: the squash
   norm over t is a 0/1-selector matmul, the per-(c,token) scale is
   replicated back over t with a second tiny matmul, and sem is written to
   DRAM with contiguous 2KB runs (48 descriptors/write instead of 768 —
   HWDGE descriptor generation was the old phase-1 pacing bottleneck).
 - x loaded token-chunk-major, pipelined with the phase-1 matmuls.
 - rw prefetched right after phase 1 (explicit dep) so x gets the full
   HBM pipe first and rw streams during the AllGather window.
 - a tiny warm-up AllGather at t=0 absorbs the first-collective setup.
 - final-linear bias folded into the matmul as a 9th contraction row.
"""

import sys

import numpy as np

if "/opt/trn_rl_repo" not in sys.path:
    sys.path.insert(0, "/opt/trn_rl_repo")

NTASKS = 10
CAP = 8
L = 256
D = 768
B = 64
N_CORES = 8
BL = B // N_CORES          # batches per core (8)
TOK = BL * L               # tokens per core (2048)
KT = D // 128              # k tiles over D (6)
IT = (L * CAP) // 128      # i tiles over L*CAP (16)
NT = TOK // 512            # phase-1 moving chunks (4)

_CACHE = {}


def _build(A, use_cc=True):
    """Build the 8-core SPMD Bass program for A = eval_t+1 active tasks."""
    import concourse.bass as bass
    import concourse.tile as tile
    import concourse.mybir as mybir
    from concourse import bacc
    from concourse.tile import add_dep_helper

    f32 = mybir.dt.float32
    f32r = mybir.dt.float32r
    Alu = mybir.AluOpType
    Act = mybir.ActivationFunctionType
    X = mybir.AxisListType.X

    nc = bacc.Bacc("TRN2", target_bir_lowering=False, debug=False,
                   num_devices=N_CORES)

    TC = NTASKS * CAP  # 80
    AC = A * CAP
    NPAIR = (A + 1) // 2   # task-pair transpose tiles
    RLO = min(A, 4)        # tasks in collective chunk 0
    CH = [RLO, A - RLO] if A > RLO else [A]   # tasks per chunk
    CH0 = [0, RLO]

    xT = nc.dram_tensor("xT", [D, TOK], f32r, kind="ExternalInput").ap()
    weffT = nc.dram_tensor("weffT", [D, TC], f32r, kind="ExternalInput").ap()
    beff_col = nc.dram_tensor("beff_col", [TC, 1], f32,
                              kind="ExternalInput").ap()
    # rw_h[p, (r, k, o)] = route_weights[core, r, i2(k,p), o] where the
    # contraction index is reordered to i2 = c*L + l (phase-1 sem rows are
    # (t, c) with token cols, so gathered sem transposes to (c, l) order)
    rw = nc.dram_tensor("rw", [128, A * IT * L], f32r,
                        kind="ExternalInput").ap()
    # wlT9 = [larger_w[e].T; larger_b[e]] -- bias folded in as a 9th
    # contraction row so phase 6 needs no separate bias add
    wlT9 = nc.dram_tensor("wlT9", [CAP + 1, D], f32r,
                          kind="ExternalInput").ap()
    ones_row = nc.dram_tensor("ones_row", [1, 32 * B], f32,
                              kind="ExternalInput").ap()
    # squash helpers: selT[(t,c), c'] = (c == c'); repT[c, (t<A,c')] = (c==c')
    selT = nc.dram_tensor("selT", [TC, CAP], f32r,
                          kind="ExternalInput").ap()
    repT = nc.dram_tensor("repT", [CAP, AC], f32r,
                          kind="ExternalInput").ap()
    ident = nc.dram_tensor("ident", [128, 128], f32, kind="ExternalInput").ap()
    out = nc.dram_tensor("out", [BL, L, D], f32, kind="ExternalOutput").ap()

    # collective chunks by task: rows (t, c) t-major, cols (b_l, l)
    sem_p = [nc.dram_tensor(f"sem_p{i}", [n * CAP, TOK], f32).ap()
             for i, n in enumerate(CH)]
    gath_p = [nc.dram_tensor(f"gath_p{i}", [N_CORES * n * CAP, TOK], f32,
                             addr_space="Shared").ap()
              for i, n in enumerate(CH)]
    # tiny collective to absorb the first-op ncfw/channel setup cost
    # while phase 1 is still computing
    cc_warm_in = nc.dram_tensor("cc_warm_in", [1, 16], f32).ap()
    cc_warm_out = nc.dram_tensor("cc_warm_out", [N_CORES, 16], f32,
                                 addr_space="Shared").ap()
    voteT_dram = nc.dram_tensor("voteT_dram", [L, B], f32).ap()

    with tile.TileContext(nc) as tc:
        with tc.tile_pool(name="singles", bufs=1) as singles:
            # ---- constants ----
            weff_sb = singles.tile([128, KT * TC], f32r)
            nc.sync.dma_start(out=weff_sb,
                              in_=weffT.rearrange("(k p) c -> p k c", p=128))
            beff_sb = singles.tile([TC, 1], f32)
            nc.sync.dma_start(out=beff_sb, in_=beff_col)
            ident_sb = singles.tile([128, 128], f32)
            nc.sync.dma_start(out=ident_sb, in_=ident)
            wlT_sb = singles.tile([CAP + 1, D], f32r)
            nc.sync.dma_start(out=wlT_sb, in_=wlT9)
            sel_sb = singles.tile([TC, CAP], f32r)
            nc.sync.dma_start(out=sel_sb, in_=selT)
            rep_sb = singles.tile([CAP, AC], f32r)
            nc.sync.dma_start(out=rep_sb, in_=repT)

            priors_sb = singles.tile([64, A * L], f32)
            semT_sb = singles.tile([128, NPAIR * IT * 128], f32r)

            rw_sb = []
            for r in range(A):
                rwt = singles.tile([128, IT * L], f32r, tag=f"rw{r}")
                rw_sb.append(rwt)

            # ===== Phase 1: semantic stage, batch-parallel ================
            # All in the [(t,c), token] layout h2 is produced in:
            #   h2a[80, 512] (+bias, DVE); h2sq = h2a^2 (DVE);
            #   sq[c, tok] = selT.T @ h2sq (PE); scal = sqrt(sq)/(1+sq)
            #   with 1/(1+sq) = exp(-ln(1+sq)) -- ACT ops batched by
            #   function so table reloads (~1.3us each) happen ~3x total;
            #   scal_rep = repT.T @ scal (PE); sem = h2a[:AC] * scal_rep
            #   (DVE) -> contiguous DRAM write (2KB runs).
            with (
                tc.tile_pool(name="x_pool", bufs=8) as xpool,
                tc.tile_pool(name="pA", bufs=2, space="PSUM") as pA,
                tc.tile_pool(name="pS", bufs=4, space="PSUM") as pS,
                tc.tile_pool(name="pR", bufs=2, space="PSUM") as pR,
                tc.tile_pool(name="h2a_pool", bufs=4) as hapool,
                tc.tile_pool(name="sem_pool", bufs=2) as spool,
                tc.tile_pool(name="sq_pool", bufs=4) as qpool,
            ):
                h2as, psqs, rts, dens, lnds, rdens, scals = \
                    [], [], [], [], [], [], []
                for nt in range(NT):            # 4 chunks of 512 tokens
                    xks = []
                    for k in range(KT):
                        xk = xpool.tile([128, 512], f32r, tag="xk")
                        nc.sync.dma_start(
                            out=xk,
                            in_=xT[k * 128:(k + 1) * 128,
                                   nt * 512:(nt + 1) * 512])
                        xks.append(xk)
                    psa = pA.tile([TC, 512], f32, tag="psa")
                    for k in range(KT):
                        nc.tensor.matmul(
                            psa,
                            lhsT=weff_sb[:, k * TC:(k + 1) * TC],
                            rhs=xks[k],
                            start=(k == 0), stop=(k == KT - 1),
                        )
                    h2a = hapool.tile([TC, 512], f32, tag="h2a")
                    nc.vector.tensor_scalar_add(h2a, psa, beff_sb)
                    h2sq = spool.tile([TC, 512], f32r, tag="h2sq")
                    nc.vector.tensor_mul(h2sq, h2a, h2a)
                    psq = pS.tile([CAP, 512], f32, tag="psq")
                    nc.tensor.matmul(psq, lhsT=sel_sb, rhs=h2sq,
                                     start=True, stop=True)
                    h2as.append(h2a)
                    psqs.append(psq)
                for nt in range(NT):
                    rt = qpool.tile([CAP, 512], f32, tag="rt")
                    nc.scalar.activation(rt, psqs[nt], Act.Sqrt)
                    rts.append(rt)
                for nt in range(NT):
                    den = qpool.tile([CAP, 512], f32, tag="den")
                    nc.vector.tensor_scalar_add(den, psqs[nt], 1.0)
                    dens.append(den)
                for nt in range(NT):
                    lnd = qpool.tile([CAP, 512], f32, tag="lnd")
                    nc.scalar.activation(lnd, dens[nt], Act.Ln)
                    lnds.append(lnd)
                for nt in range(NT):
                    rden = qpool.tile([CAP, 512], f32, tag="rden")
                    nc.scalar.activation(rden, lnds[nt], Act.Exp,
                                         scale=-1.0)
                    rdens.append(rden)
                last_sem_write = None
                for nt in range(NT):
                    scal = qpool.tile([CAP, 512], f32r, tag="scal")
                    nc.vector.tensor_mul(scal, rts[nt], rdens[nt])
                    prep = pR.tile([AC, 512], f32, tag="prep")
                    nc.tensor.matmul(prep, lhsT=rep_sb, rhs=scal,
                                     start=True, stop=True)
                    sem = spool.tile([AC, 512], f32, tag="sem")
                    nc.vector.tensor_tensor(out=sem, in0=h2as[nt][:AC],
                                            in1=prep, op=Alu.mult)
                    for i, n in enumerate(CH):
                        wr = nc.sync.dma_start(
                            out=sem_p[i][:, nt * 512:(nt + 1) * 512],
                            in_=sem[CH0[i] * CAP:(CH0[i] + n) * CAP])
                        last_sem_write = wr

            # ---- rw prefetch: emitted on the sync ring AFTER the phase-1
            # x loads, so x descriptors drain first and rw streams during
            # the AllGather window ----
            for r in range(A):
                nc.sync.dma_start(
                    out=rw_sb[r], in_=rw[:, r * IT * L:(r + 1) * IT * L])

            # ---- PE keep-warm through the AllGather window (~40us of
            # back-to-back dummy matmuls) so phase 3/4 runs at 2.4 GHz ----
            with tc.tile_pool(name="pW", bufs=2, space="PSUM") as pW:
                for w in range(160):
                    pdw = pW.tile([TC, 512], f32, tag="pdw")
                    mm = nc.tensor.matmul(
                        pdw[:, 0:448], lhsT=weff_sb[:, 0:TC],
                        rhs=weff_sb[:, 0:448],
                        start=True, stop=True)
                    if w == 0:
                        add_dep_helper(last_sem_write.ins, mm.ins,
                                       sync=True,
                                       reason="warm PE during AllGather")

            # ===== Phase 2: allgather sem (task chunks) ===================
            if use_cc:
                for i in range(len(CH)):
                    nc.gpsimd.collective_compute(
                        "AllGather", Alu.bypass,
                        replica_groups=[list(range(N_CORES))],
                        ins=[sem_p[i][:]], outs=[gath_p[i][:]])
            else:
                for i in range(len(CH)):
                    nc.sync.dma_start(out=gath_p[i][0:CH[i] * CAP],
                                      in_=sem_p[i][:])

            # ===== Phase 3+4: gather-transpose + priors ===================
            # Pair tiles: tile t holds tasks (2t, 2t+1) x all 64 batches,
            # partition = (task, rank, b_l), cols = (c, l); PE transposes
            # give semT[(c,l)-slice, (task, batch)] for the priors lhsT.
            NP = NPAIR

            def chunk_r(r):
                return (0, r) if r < RLO else (1, r - RLO)

            with (
                tc.tile_pool(name="gpool", bufs=2) as gpool,
                tc.tile_pool(name="pT", bufs=4, space="PSUM") as pT,
                tc.tile_pool(name="pP", bufs=3, space="PSUM") as pP,
            ):
                g_tiles = []
                for t in range(NP):
                    g_sb = gpool.tile([128, L * CAP], f32, tag="g")
                    for ri in range(2):
                        if 2 * t + ri >= A:
                            continue
                        ci, rloc = chunk_r(2 * t + ri)
                        for rank in range(N_CORES):
                            base = (rank * CH[ci] + rloc) * CAP
                            eng = nc.sync if (rank % 2 == 0) else nc.scalar
                            eng.dma_start(
                                out=g_sb[ri * 64 + rank * 8:
                                         ri * 64 + rank * 8 + 8].rearrange(
                                    "p (c l) -> p c l", c=CAP),
                                in_=gath_p[ci][base:base + CAP, :].rearrange(
                                    "c (b l) -> b c l", b=BL))
                    g_tiles.append(g_sb)

                for t in range(NP):
                    for k in range(IT):
                        psT = pT.tile([128, 128], f32, tag="psT")
                        nc.tensor.transpose(
                            psT, in_=g_tiles[t][:, k * 128:(k + 1) * 128],
                            identity=ident_sb)
                        cp = nc.vector if (k % 3) else nc.scalar
                        dst = semT_sb[:, (t * IT + k) * 128:
                                      (t * IT + k + 1) * 128]
                        if cp is nc.vector:
                            nc.vector.tensor_copy(out=dst, in_=psT)
                        else:
                            nc.scalar.activation(dst, psT, Act.Copy)
                    for ri in range(2):
                        r = 2 * t + ri
                        if r >= A:
                            continue
                        pp = pP.tile([64, L], f32, tag="pp")
                        for k in range(IT):
                            base = (t * IT + k) * 128 + ri * 64
                            nc.tensor.matmul(
                                pp, lhsT=semT_sb[:, base:base + 64],
                                rhs=rw_sb[r][:, k * L:(k + 1) * L],
                                start=(k == 0), stop=(k == IT - 1))
                        cp = nc.vector if (r % 2 == 0) else nc.scalar
                        dst = priors_sb[:, r * L:(r + 1) * L]
                        if cp is nc.vector:
                            nc.vector.tensor_copy(out=dst, in_=pp)
                        else:
                            nc.scalar.activation(dst, pp, Act.Copy)

            # ===== Phase 5: routing (vectorized over r) ===================
            with (
                tc.tile_pool(name="route", bufs=1) as rp,
                tc.tile_pool(name="pV", bufs=2, space="PSUM") as pV,
            ):
                vote = rp.tile([64, L], f32)
                scr = rp.tile([64, L], f32)
                big = rp.tile([64, A * L], f32)
                l1 = rp.tile([64, A], f32)
                l2 = rp.tile([64, A], f32)
                dots_raw = rp.tile([64, A], f32)
                dots = rp.tile([64, A], f32)
                ex = rp.tile([64, A], f32)
                probs = rp.tile([64, A], f32)
                n2 = rp.tile([64, 1], f32)
                rt2 = rp.tile([64, 1], f32)
                den2 = rp.tile([64, 1], f32)
                rden2 = rp.tile([64, 1], f32)
                sc2 = rp.tile([64, 1], f32)
                mx = rp.tile([64, 1], f32)
                nmx = rp.tile([64, 1], f32)
                ssum = rp.tile([64, 1], f32)
                rsum = rp.tile([64, 1], f32)

                def warm(dep, m):
                    # tiny matmul with a true dep on the routing chain --
                    # keeps the PE HAM un-throttled through phase 5
                    pdum = pV.tile([64, 128], f32, tag="pdum")
                    nc.tensor.matmul(pdum[:m], lhsT=dep[:, 0:m],
                                     rhs=priors_sb[:, 0:128],
                                     start=True, stop=True)

                def squash_scal():
                    # sc2 = sqrt(n2)/(1+n2); outsq = sc2*vote is never
                    # materialized -- dots get scaled by sc2 instead.
                    nc.vector.tensor_mul(scr, vote, vote)
                    nc.vector.tensor_reduce(out=n2, in_=scr, axis=X,
                                            op=Alu.add)
                    nc.scalar.activation(rt2, n2, Act.Sqrt)
                    nc.vector.tensor_scalar_add(den2, n2, 1.0)
                    nc.vector.reciprocal(rden2, den2)
                    nc.vector.tensor_mul(sc2, rt2, rden2)

                def logit_update(l_prev, l_new):
                    for r in range(A):
                        nc.vector.scalar_tensor_tensor(
                            out=big[:, r * L:(r + 1) * L],
                            in0=priors_sb[:, r * L:(r + 1) * L],
                            scalar=1.0, in1=vote,
                            op0=Alu.mult, op1=Alu.mult,
                            accum_out=dots_raw[:, r:r + 1])
                    if l_prev is None:
                        nc.vector.tensor_scalar_mul(l_new, dots_raw, sc2)
                    else:
                        nc.vector.tensor_scalar_mul(dots, dots_raw, sc2)
                        nc.vector.tensor_add(l_new, dots, l_prev)

                def softmax_vote(l_in):
                    nc.vector.tensor_reduce(out=mx, in_=l_in, axis=X,
                                            op=Alu.max)
                    nc.vector.tensor_scalar_mul(nmx, mx, -1.0)
                    nc.scalar.activation(ex, l_in, Act.Exp, bias=nmx,
                                         accum_out=ssum)
                    nc.vector.reciprocal(rsum, ssum)
                    nc.vector.tensor_scalar_mul(probs, ex, rsum)
                    pr_b = bass.AP(
                        tensor=probs.tensor, offset=probs.offset,
                        ap=[probs.ap[0], [1, A], [0, L]])
                    nc.vector.tensor_tensor(
                        out=big.rearrange("p (r o) -> p r o", r=A),
                        in0=priors_sb.rearrange("p (r o) -> p r o", r=A),
                        in1=pr_b, op=Alu.mult)
                    nc.vector.tensor_reduce(
                        out=vote,
                        in_=big.rearrange("p (r o) -> p o r", r=A),
                        axis=X, op=Alu.add)

                # iter 1: uniform probs = 1/A
                nc.vector.tensor_reduce(
                    out=scr,
                    in_=priors_sb.rearrange("p (r o) -> p o r", r=A),
                    axis=X, op=Alu.add)
                nc.vector.tensor_scalar_mul(vote, scr, 1.0 / A)
                squash_scal()
                warm(vote, 64)
                logit_update(None, l1)
                warm(l1, A)
                softmax_vote(l1)
                warm(vote, 64)
                squash_scal()
                logit_update(l1, l2)
                warm(l2, A)
                softmax_vote(l2)

                # transpose vote [64, 256] -> voteT_dram [256, 64]
                vT_sb = rp.tile([128, 128], f32)
                for half in range(2):
                    pv = pV.tile([128, 64], f32, tag="pv")
                    nc.tensor.transpose(
                        pv, in_=vote[:, half * 128:(half + 1) * 128],
                        identity=ident_sb[:64, :64])
                    nc.vector.tensor_copy(
                        out=vT_sb[:, half * 64:(half + 1) * 64], in_=pv)
                    nc.sync.dma_start(
                        out=voteT_dram[half * 128:(half + 1) * 128],
                        in_=vT_sb[:, half * 64:(half + 1) * 64])

            # ===== Phase 6: final linear ==================================
            # voteT_dram[o, b]; h_blT[cap, l] = voteT[(l%32)*8+cap,
            # b_l*8 + l//32].  vt2[cap, (lr, b)] loads with 256B bursts;
            # row CAP is all-ones so wlT9's bias row lands in the matmul.
            with (
                tc.tile_pool(name="vt", bufs=1) as vtp,
                tc.tile_pool(name="pF", bufs=4, space="PSUM") as pF,
                tc.tile_pool(name="outp", bufs=3) as op_,
            ):
                vt2 = vtp.tile([CAP + 1, 32 * B], f32)
                src = bass.AP(
                    tensor=voteT_dram.tensor, offset=voteT_dram.offset,
                    ap=[[B, CAP], [CAP * B, 32], [1, B]])
                nc.sync.dma_start(out=vt2[:CAP], in_=src)
                nc.sync.dma_start(out=vt2[CAP:CAP + 1], in_=ones_row)
                # permute free layout (lr, b) -> (b, lr) during the f32r
                # convert, so each lhsT is a contiguous 128-col slice
                vt2r = vtp.tile([CAP + 1, 32 * B], f32r)
                nc.vector.tensor_copy(
                    out=vt2r.rearrange("p (b lr) -> p b lr", lr=32),
                    in_=vt2.rearrange("p (lr b) -> p b lr", lr=32))
                NH = 2
                for b_l in range(BL):
                    for lt in range(2):
                        o_sb = op_.tile([128, D], f32, tag="o")
                        lhsT = vt2r[:, (b_l * CAP + lt * 4) * 32:
                                    (b_l * CAP + lt * 4) * 32 + 128]
                        for nh in range(NH):
                            pf = pF.tile([128, D // NH], f32, tag="pf")
                            nc.tensor.matmul(
                                pf, lhsT=lhsT,
                                rhs=wlT_sb[:, nh * (D // NH):
                                           (nh + 1) * (D // NH)],
                                start=True, stop=True)
                            dst = o_sb[:, nh * (D // NH):(nh + 1) * (D // NH)]
                            if nh == 0:
                                nc.vector.tensor_copy(out=dst, in_=pf)
                            else:
                                nc.scalar.activation(dst, pf, Act.Copy)
                        nc.sync.dma_start(
                            out=out[b_l, lt * 128:(lt + 1) * 128, :],
                            in_=o_sb)

    nc.compile()
    return nc


def _host_prep(x, fc1_w, fc1_b, fc2_w, fc2_b, route_weights, larger_w,
               larger_b, eval_t):
    A = int(eval_t) + 1
    f64 = np.float64
    weff = np.einsum("tcd,tdi->tci", fc2_w.astype(f64), fc1_w.astype(f64))
    beff = (np.einsum("tcd,td->tc", fc2_w.astype(f64), fc1_b.astype(f64))
            + fc2_b.astype(f64))
    weffT = np.ascontiguousarray(
        weff.reshape(NTASKS * CAP, D).T).astype(np.float32)
    beff_col = beff.reshape(NTASKS * CAP, 1).astype(np.float32)
    wlT9 = np.ascontiguousarray(np.concatenate(
        [larger_w[int(eval_t)].T, larger_b[int(eval_t)].reshape(1, D)],
        axis=0)).astype(np.float32)
    ones_row = np.ones((1, 32 * B), dtype=np.float32)
    selT = np.tile(np.eye(CAP, dtype=np.float32), (NTASKS, 1))
    repT = np.tile(np.eye(CAP, dtype=np.float32), (1, A))
    ident = np.eye(128, dtype=np.float32)

    in_maps = []
    for c in range(N_CORES):
        xT_c = np.ascontiguousarray(
            x[c * BL:(c + 1) * BL].reshape(TOK, D).T).astype(np.float32)
        # reorder the contraction index to i2 = c2*L + l, then k-tile:
        # rw_c[p, (r, k, o)] = route_weights[c, r, l(k,p)*CAP + c2(k,p), o]
        rw2 = route_weights[c, :A].reshape(A, L, CAP, L).transpose(0, 2, 1, 3)
        rw_c = np.ascontiguousarray(
            rw2.reshape(A, IT, 128, L)
            .transpose(2, 0, 1, 3).reshape(128, A * IT * L)).astype(
                np.float32)
        in_maps.append({
            "xT": xT_c, "weffT": weffT, "beff_col": beff_col, "rw": rw_c,
            "wlT9": wlT9, "ones_row": ones_row, "selT": selT, "repT": repT,
            "ident": ident,
        })
    return A, in_maps


def kernel(**inputs):
    from concourse.bass_utils import run_bass_kernel_spmd

    A, in_maps = _host_prep(**inputs)
    if A not in _CACHE:
        _CACHE[A] = _build(A)
    nc = _CACHE[A]
    res = run_bass_kernel_spmd(nc, in_maps, core_ids=list(range(N_CORES)))
    return np.concatenate(
        [res.results[c]["out"] for c in range(N_CORES)], axis=0)


# revision 41
# speedup vs baseline: 1.1436x; 1.1128x over previous
"""CapsNet-BCL Trainium2 kernel: 8-core SPMD Bass/Tile implementation.

Host algebra: fc1/fc2 have no nonlinearity between them, so
Weff[t] = fc2_w[t] @ fc1_w[t], beff[t] = fc2_w[t]@fc1_b[t]+fc2_b[t] and
h2 = x @ Weff[t].T + beff[t].  Only tasks r <= eval_t route (softmax mask
-10000 underflows to exactly 0 in fp32), so only route_weights[:, :eval_t+1]
is read.

Sharding: core k computes h2/sem for batches [8k, 8k+8); sem is AllGathered
in two task chunks ({r0..3} then {r4..}); core c computes priors+routing for
capsule c over all 64 batches.  The torch flat view vote(CAP,B,1,L)->
(B,L,CAP) maps output batch b to vote capsule b//8, so core c's vote is
exactly what output batches [8c,8c+8) need: each core emits its own output
slice, no second collective.

Numerics: the routing softmax saturates (|logits| to ~200, top-2 gaps down
to ~2.5), so priors need ~1e-4 relative accuracy — everything in the priors
path stays f32/f32r.

Perf structure vs the original baseline:
 - phase 1 stays in the matmul's natural [(t,c), token] layout: the squash
   norm over t is a 0/1-selector matmul, the per-(c,token) scale is
   replicated back over t with a second tiny matmul, and sem is written to
   DRAM with contiguous 2KB runs (48 descriptors/write instead of 768 —
   HWDGE descriptor generation was the old phase-1 pacing bottleneck).
 - x loaded token-chunk-major, pipelined with the phase-1 matmuls.
 - rw prefetched right after phase 1 (explicit dep) so x gets the full
   HBM pipe first and rw streams during the AllGather window.
 - a tiny warm-up AllGather at t=0 absorbs the first-collective setup.
 - final-linear bias folded into the matmul as a 9th contraction row.
"""

import sys

import numpy as np

if "/opt/trn_rl_repo" not in sys.path:
    sys.path.insert(0, "/opt/trn_rl_repo")

NTASKS = 10
CAP = 8
L = 256
D = 768
B = 64
N_CORES = 8
BL = B // N_CORES          # batches per core (8)
TOK = BL * L               # tokens per core (2048)
KT = D // 128              # k tiles over D (6)
IT = (L * CAP) // 128      # i tiles over L*CAP (16)
NT = TOK // 512            # phase-1 moving chunks (4)

_CACHE = {}


def _build(A, use_cc=True):
    """Build the 8-core SPMD Bass program for A = eval_t+1 active tasks."""
    import concourse.bass as bass
    import concourse.tile as tile
    import concourse.mybir as mybir
    from concourse import bacc
    from concourse.tile import add_dep_helper

    f32 = mybir.dt.float32
    f32r = mybir.dt.float32r
    Alu = mybir.AluOpType
    Act = mybir.ActivationFunctionType
    X = mybir.AxisListType.X

    nc = bacc.Bacc("TRN2", target_bir_lowering=False, debug=False,
                   num_devices=N_CORES)

    TC = NTASKS * CAP  # 80
    AC = A * CAP
    NPAIR = (A + 1) // 2   # task-pair transpose tiles
    RLO = min(A, 4)        # tasks in collective chunk 0
    CH = [RLO, A - RLO] if A > RLO else [A]   # tasks per chunk
    CH0 = [0, RLO]

    xT = nc.dram_tensor("xT", [D, TOK], f32r, kind="ExternalInput").ap()
    weffT = nc.dram_tensor("weffT", [D, TC], f32r, kind="ExternalInput").ap()
    beff_col = nc.dram_tensor("beff_col", [TC, 1], f32,
                              kind="ExternalInput").ap()
    # rw_h[p, (r, k, o)] = route_weights[core, r, i2(k,p), o] where the
    # contraction index is reordered to i2 = c*L + l (phase-1 sem rows are
    # (t, c) with token cols, so gathered sem transposes to (c, l) order)
    rw = nc.dram_tensor("rw", [128, A * IT * L], f32r,
                        kind="ExternalInput").ap()
    # wlT9 = [larger_w[e].T; larger_b[e]] -- bias folded in as a 9th
    # contraction row so phase 6 needs no separate bias add
    wlT9 = nc.dram_tensor("wlT9", [CAP + 1, D], f32r,
                          kind="ExternalInput").ap()
    ones_row = nc.dram_tensor("ones_row", [1, 32 * B], f32,
                              kind="ExternalInput").ap()
    # squash helpers: selT[(t,c), c'] = (c == c'); repT[c, (t<A,c')] = (c==c')
    selT = nc.dram_tensor("selT", [TC, CAP], f32r,
                          kind="ExternalInput").ap()
    repT = nc.dram_tensor("repT", [CAP, AC], f32r,
                          kind="ExternalInput").ap()
    ident = nc.dram_tensor("ident", [128, 128], f32, kind="ExternalInput").ap()
    out = nc.dram_tensor("out", [BL, L, D], f32, kind="ExternalOutput").ap()

    # collective chunks by task: rows (t, c) t-major, cols (b_l, l)
    sem_p = [nc.dram_tensor(f"sem_p{i}", [n * CAP, TOK], f32).ap()
             for i, n in enumerate(CH)]
    gath_p = [nc.dram_tensor(f"gath_p{i}", [N_CORES * n * CAP, TOK], f32,
                             addr_space="Shared").ap()
              for i, n in enumerate(CH)]
    # tiny 2-rank collective (single algorithm step) to absorb the
    # first-op ncfw reaction cost while phase 1 is still computing
    cc_warm_in = nc.dram_tensor("cc_warm_in", [1, 16], f32).ap()
    cc_warm_out = nc.dram_tensor("cc_warm_out", [4, 16], f32,
                                 addr_space="Shared").ap()
    voteT_dram = nc.dram_tensor("voteT_dram", [L, B], f32).ap()

    with tile.TileContext(nc) as tc:
        with tc.tile_pool(name="singles", bufs=1) as singles:
            # ---- constants ----
            weff_sb = singles.tile([128, KT * TC], f32r)
            nc.sync.dma_start(out=weff_sb,
                              in_=weffT.rearrange("(k p) c -> p k c", p=128))
            beff_sb = singles.tile([TC, 1], f32)
            nc.sync.dma_start(out=beff_sb, in_=beff_col)
            ident_sb = singles.tile([128, 128], f32)
            nc.sync.dma_start(out=ident_sb, in_=ident)
            wlT_sb = singles.tile([CAP + 1, D], f32r)
            nc.sync.dma_start(out=wlT_sb, in_=wlT9)
            sel_sb = singles.tile([TC, CAP], f32r)
            nc.sync.dma_start(out=sel_sb, in_=selT)
            rep_sb = singles.tile([CAP, AC], f32r)
            nc.sync.dma_start(out=rep_sb, in_=repT)

            priors_sb = singles.tile([64, A * L], f32)
            semT_sb = singles.tile([128, NPAIR * IT * 128], f32r)

            rw_sb = []
            for r in range(A):
                rwt = singles.tile([128, IT * L], f32r, tag=f"rw{r}")
                rw_sb.append(rwt)

            # ===== Phase 1: semantic stage, batch-parallel ================
            # All in the [(t,c), token] layout h2 is produced in:
            #   h2a[80, 512] (+bias, DVE); h2sq = h2a^2 (DVE);
            #   sq[c, tok] = selT.T @ h2sq (PE); scal = sqrt(sq)/(1+sq)
            #   with 1/(1+sq) = exp(-ln(1+sq)) -- ACT ops batched by
            #   function so table reloads (~1.3us each) happen ~3x total;
            #   scal_rep = repT.T @ scal (PE); sem = h2a[:AC] * scal_rep
            #   (DVE) -> contiguous DRAM write (2KB runs).
            with (
                tc.tile_pool(name="x_pool", bufs=8) as xpool,
                tc.tile_pool(name="pA", bufs=2, space="PSUM") as pA,
                tc.tile_pool(name="pS", bufs=4, space="PSUM") as pS,
                tc.tile_pool(name="pR", bufs=2, space="PSUM") as pR,
                tc.tile_pool(name="h2a_pool", bufs=4) as hapool,
                tc.tile_pool(name="sem_pool", bufs=2) as spool,
                tc.tile_pool(name="sq_pool", bufs=4) as qpool,
            ):
                h2as, psqs, rts, dens, lnds, rdens, scals = \
                    [], [], [], [], [], [], []
                for nt in range(NT):            # 4 chunks of 512 tokens
                    xks = []
                    for k in range(KT):
                        xk = xpool.tile([128, 512], f32r, tag="xk")
                        nc.sync.dma_start(
                            out=xk,
                            in_=xT[k * 128:(k + 1) * 128,
                                   nt * 512:(nt + 1) * 512])
                        xks.append(xk)
                    psa = pA.tile([TC, 512], f32, tag="psa")
                    for k in range(KT):
                        nc.tensor.matmul(
                            psa,
                            lhsT=weff_sb[:, k * TC:(k + 1) * TC],
                            rhs=xks[k],
                            start=(k == 0), stop=(k == KT - 1),
                        )
                    h2a = hapool.tile([TC, 512], f32, tag="h2a")
                    nc.vector.tensor_scalar_add(h2a, psa, beff_sb)
                    h2sq = spool.tile([TC, 512], f32r, tag="h2sq")
                    nc.vector.tensor_mul(h2sq, h2a, h2a)
                    psq = pS.tile([CAP, 512], f32, tag="psq")
                    nc.tensor.matmul(psq, lhsT=sel_sb, rhs=h2sq,
                                     start=True, stop=True)
                    h2as.append(h2a)
                    psqs.append(psq)
                for nt in range(NT):
                    rt = qpool.tile([CAP, 512], f32, tag="rt")
                    nc.scalar.activation(rt, psqs[nt], Act.Sqrt)
                    rts.append(rt)
                for nt in range(NT):
                    den = qpool.tile([CAP, 512], f32, tag="den")
                    nc.vector.tensor_scalar_add(den, psqs[nt], 1.0)
                    dens.append(den)
                for nt in range(NT):
                    lnd = qpool.tile([CAP, 512], f32, tag="lnd")
                    nc.scalar.activation(lnd, dens[nt], Act.Ln)
                    lnds.append(lnd)
                for nt in range(NT):
                    rden = qpool.tile([CAP, 512], f32, tag="rden")
                    nc.scalar.activation(rden, lnds[nt], Act.Exp,
                                         scale=-1.0)
                    rdens.append(rden)
                last_sem_write = None
                for nt in range(NT):
                    scal = qpool.tile([CAP, 512], f32r, tag="scal")
                    nc.vector.tensor_mul(scal, rts[nt], rdens[nt])
                    prep = pR.tile([AC, 512], f32, tag="prep")
                    nc.tensor.matmul(prep, lhsT=rep_sb, rhs=scal,
                                     start=True, stop=True)
                    sem = spool.tile([AC, 512], f32, tag="sem")
                    nc.vector.tensor_tensor(out=sem, in0=h2as[nt][:AC],
                                            in1=prep, op=Alu.mult)
                    for i, n in enumerate(CH):
                        wr = nc.sync.dma_start(
                            out=sem_p[i][:, nt * 512:(nt + 1) * 512],
                            in_=sem[CH0[i] * CAP:(CH0[i] + n) * CAP])
                        last_sem_write = wr

            # ---- rw prefetch: emitted on the sync ring AFTER the phase-1
            # x loads, so x descriptors drain first and rw streams during
            # the AllGather window ----
            for r in range(A):
                nc.sync.dma_start(
                    out=rw_sb[r], in_=rw[:, r * IT * L:(r + 1) * IT * L])

            # ---- PE keep-warm through the AllGather window (~40us of
            # back-to-back dummy matmuls) so phase 3/4 runs at 2.4 GHz ----
            with tc.tile_pool(name="pW", bufs=2, space="PSUM") as pW:
                for w in range(100):
                    pdw = pW.tile([TC, 512], f32, tag="pdw")
                    mm = nc.tensor.matmul(
                        pdw[:, 0:448], lhsT=weff_sb[:, 0:TC],
                        rhs=weff_sb[:, 0:448],
                        start=True, stop=True)
                    if w == 0:
                        add_dep_helper(last_sem_write.ins, mm.ins,
                                       sync=True,
                                       reason="warm PE during AllGather")

            # ===== Phase 2: allgather sem (task chunks) ===================
            if use_cc:
                for i in range(len(CH)):
                    nc.gpsimd.collective_compute(
                        "AllGather", Alu.bypass,
                        replica_groups=[list(range(N_CORES))],
                        ins=[sem_p[i][:]], outs=[gath_p[i][:]])
            else:
                for i in range(len(CH)):
                    nc.sync.dma_start(out=gath_p[i][0:CH[i] * CAP],
                                      in_=sem_p[i][:])

            # ===== Phase 3+4: gather-transpose + priors ===================
            # Pair tiles: tile t holds tasks (2t, 2t+1) x all 64 batches,
            # partition = (task, rank, b_l), cols = (c, l); PE transposes
            # give semT[(c,l)-slice, (task, batch)] for the priors lhsT.
            NP = NPAIR

            def chunk_r(r):
                return (0, r) if r < RLO else (1, r - RLO)

            with (
                tc.tile_pool(name="gpool", bufs=2) as gpool,
                tc.tile_pool(name="pT", bufs=4, space="PSUM") as pT,
                tc.tile_pool(name="pP", bufs=3, space="PSUM") as pP,
            ):
                g_tiles = []
                for t in range(NP):
                    g_sb = gpool.tile([128, L * CAP], f32, tag="g")
                    for ri in range(2):
                        if 2 * t + ri >= A:
                            continue
                        ci, rloc = chunk_r(2 * t + ri)
                        for rank in range(N_CORES):
                            base = (rank * CH[ci] + rloc) * CAP
                            eng = nc.sync if (rank % 2 == 0) else nc.scalar
                            eng.dma_start(
                                out=g_sb[ri * 64 + rank * 8:
                                         ri * 64 + rank * 8 + 8].rearrange(
                                    "p (c l) -> p c l", c=CAP),
                                in_=gath_p[ci][base:base + CAP, :].rearrange(
                                    "c (b l) -> b c l", b=BL))
                    g_tiles.append(g_sb)

                for t in range(NP):
                    for k in range(IT):
                        psT = pT.tile([128, 128], f32, tag="psT")
                        nc.tensor.transpose(
                            psT, in_=g_tiles[t][:, k * 128:(k + 1) * 128],
                            identity=ident_sb)
                        cp = nc.vector if (k % 3) else nc.scalar
                        dst = semT_sb[:, (t * IT + k) * 128:
                                      (t * IT + k + 1) * 128]
                        if cp is nc.vector:
                            nc.vector.tensor_copy(out=dst, in_=psT)
                        else:
                            nc.scalar.activation(dst, psT, Act.Copy)
                    for ri in range(2):
                        r = 2 * t + ri
                        if r >= A:
                            continue
                        pp = pP.tile([64, L], f32, tag="pp")
                        for k in range(IT):
                            base = (t * IT + k) * 128 + ri * 64
                            nc.tensor.matmul(
                                pp, lhsT=semT_sb[:, base:base + 64],
                                rhs=rw_sb[r][:, k * L:(k + 1) * L],
                                start=(k == 0), stop=(k == IT - 1))
                        cp = nc.vector if (r % 2 == 0) else nc.scalar
                        dst = priors_sb[:, r * L:(r + 1) * L]
                        if cp is nc.vector:
                            nc.vector.tensor_copy(out=dst, in_=pp)
                        else:
                            nc.scalar.activation(dst, pp, Act.Copy)

            # ===== Phase 5: routing (vectorized over r) ===================
            with (
                tc.tile_pool(name="route", bufs=1) as rp,
                tc.tile_pool(name="pV", bufs=2, space="PSUM") as pV,
            ):
                vote = rp.tile([64, L], f32)
                scr = rp.tile([64, L], f32)
                big = rp.tile([64, A * L], f32)
                l1 = rp.tile([64, A], f32)
                l2 = rp.tile([64, A], f32)
                dots_raw = rp.tile([64, A], f32)
                dots = rp.tile([64, A], f32)
                ex = rp.tile([64, A], f32)
                probs = rp.tile([64, A], f32)
                n2 = rp.tile([64, 1], f32)
                rt2 = rp.tile([64, 1], f32)
                den2 = rp.tile([64, 1], f32)
                rden2 = rp.tile([64, 1], f32)
                sc2 = rp.tile([64, 1], f32)
                mx = rp.tile([64, 1], f32)
                nmx = rp.tile([64, 1], f32)
                ssum = rp.tile([64, 1], f32)
                rsum = rp.tile([64, 1], f32)

                def warm(dep, m):
                    # tiny matmul with a true dep on the routing chain --
                    # keeps the PE HAM un-throttled through phase 5
                    pdum = pV.tile([64, 128], f32, tag="pdum")
                    nc.tensor.matmul(pdum[:m], lhsT=dep[:, 0:m],
                                     rhs=priors_sb[:, 0:128],
                                     start=True, stop=True)

                def squash_scal():
                    # sc2 = sqrt(n2)/(1+n2); outsq = sc2*vote is never
                    # materialized -- dots get scaled by sc2 instead.
                    nc.vector.tensor_mul(scr, vote, vote)
                    nc.vector.tensor_reduce(out=n2, in_=scr, axis=X,
                                            op=Alu.add)
                    nc.scalar.activation(rt2, n2, Act.Sqrt)
                    nc.vector.tensor_scalar_add(den2, n2, 1.0)
                    nc.vector.reciprocal(rden2, den2)
                    nc.vector.tensor_mul(sc2, rt2, rden2)

                def logit_update(l_prev, l_new):
                    for r in range(A):
                        nc.vector.scalar_tensor_tensor(
                            out=big[:, r * L:(r + 1) * L],
                            in0=priors_sb[:, r * L:(r + 1) * L],
                            scalar=1.0, in1=vote,
                            op0=Alu.mult, op1=Alu.mult,
                            accum_out=dots_raw[:, r:r + 1])
                    if l_prev is None:
                        nc.vector.tensor_scalar_mul(l_new, dots_raw, sc2)
                    else:
                        nc.vector.tensor_scalar_mul(dots, dots_raw, sc2)
                        nc.vector.tensor_add(l_new, dots, l_prev)

                def softmax_vote(l_in):
                    nc.vector.tensor_reduce(out=mx, in_=l_in, axis=X,
                                            op=Alu.max)
                    nc.vector.tensor_scalar_mul(nmx, mx, -1.0)
                    nc.scalar.activation(ex, l_in, Act.Exp, bias=nmx,
                                         accum_out=ssum)
                    nc.vector.reciprocal(rsum, ssum)
                    nc.vector.tensor_scalar_mul(probs, ex, rsum)
                    pr_b = bass.AP(
                        tensor=probs.tensor, offset=probs.offset,
                        ap=[probs.ap[0], [1, A], [0, L]])
                    nc.vector.tensor_tensor(
                        out=big.rearrange("p (r o) -> p r o", r=A),
                        in0=priors_sb.rearrange("p (r o) -> p r o", r=A),
                        in1=pr_b, op=Alu.mult)
                    nc.vector.tensor_reduce(
                        out=vote,
                        in_=big.rearrange("p (r o) -> p o r", r=A),
                        axis=X, op=Alu.add)

                # iter 1: uniform probs = 1/A
                nc.vector.tensor_reduce(
                    out=scr,
                    in_=priors_sb.rearrange("p (r o) -> p o r", r=A),
                    axis=X, op=Alu.add)
                nc.vector.tensor_scalar_mul(vote, scr, 1.0 / A)
                squash_scal()
                warm(vote, 64)
                logit_update(None, l1)
                warm(l1, A)
                softmax_vote(l1)
                warm(vote, 64)
                squash_scal()
                logit_update(l1, l2)
                warm(l2, A)
                softmax_vote(l2)

                # transpose vote [64, 256] -> voteT_dram [256, 64]
                vT_sb = rp.tile([128, 128], f32)
                for half in range(2):
                    pv = pV.tile([128, 64], f32, tag="pv")
                    nc.tensor.transpose(
                        pv, in_=vote[:, half * 128:(half + 1) * 128],
                        identity=ident_sb[:64, :64])
                    nc.vector.tensor_copy(
                        out=vT_sb[:, half * 64:(half + 1) * 64], in_=pv)
                    nc.sync.dma_start(
                        out=voteT_dram[half * 128:(half + 1) * 128],
                        in_=vT_sb[:, half * 64:(half + 1) * 64])

            # ===== Phase 6: final linear ==================================
            # voteT_dram[o, b]; h_blT[cap, l] = voteT[(l%32)*8+cap,
            # b_l*8 + l//32].  vt2[cap, (lr, b)] loads with 256B bursts;
            # row CAP is all-ones so wlT9's bias row lands in the matmul.
            with (
                tc.tile_pool(name="vt", bufs=1) as vtp,
                tc.tile_pool(name="pF", bufs=4, space="PSUM") as pF,
                tc.tile_pool(name="outp", bufs=3) as op_,
            ):
                vt2 = vtp.tile([CAP + 1, 32 * B], f32)
                src = bass.AP(
                    tensor=voteT_dram.tensor, offset=voteT_dram.offset,
                    ap=[[B, CAP], [CAP * B, 32], [1, B]])
                nc.sync.dma_start(out=vt2[:CAP], in_=src)
                nc.sync.dma_start(out=vt2[CAP:CAP + 1], in_=ones_row)
                # permute free layout (lr, b) -> (b, lr) during the f32r
                # convert, so each lhsT is a contiguous 128-col slice
                vt2r = vtp.tile([CAP + 1, 32 * B], f32r)
                nc.vector.tensor_copy(
                    out=vt2r.rearrange("p (b lr) -> p b lr", lr=32),
                    in_=vt2.rearrange("p (lr b) -> p b lr", lr=32))
                NH = 2
                for b_l in range(BL):
                    for lt in range(2):
                        o_sb = op_.tile([128, D], f32, tag="o")
                        lhsT = vt2r[:, (b_l * CAP + lt * 4) * 32:
                                    (b_l * CAP + lt * 4) * 32 + 128]
                        for nh in range(NH):
                            pf = pF.tile([128, D // NH], f32, tag="pf")
                            nc.tensor.matmul(
                                pf, lhsT=lhsT,
                                rhs=wlT_sb[:, nh * (D // NH):
                                           (nh + 1) * (D // NH)],
                                start=True, stop=True)
                            dst = o_sb[:, nh * (D // NH):(nh + 1) * (D // NH)]
                            if nh == 0:
                                nc.vector.tensor_copy(out=dst, in_=pf)
                            else:
                                nc.scalar.activation(dst, pf, Act.Copy)
                        nc.sync.dma_start(
                            out=out[b_l, lt * 128:(lt + 1) * 128, :],
                            in_=o_sb)

    nc.compile()
    return nc


def _host_prep(x, fc1_w, fc1_b, fc2_w, fc2_b, route_weights, larger_w,
               larger_b, eval_t):
    A = int(eval_t) + 1
    f64 = np.float64
    weff = np.einsum("tcd,tdi->tci", fc2_w.astype(f64), fc1_w.astype(f64))
    beff = (np.einsum("tcd,td->tc", fc2_w.astype(f64), fc1_b.astype(f64))
            + fc2_b.astype(f64))
    weffT = np.ascontiguousarray(
        weff.reshape(NTASKS * CAP, D).T).astype(np.float32)
    beff_col = beff.reshape(NTASKS * CAP, 1).astype(np.float32)
    wlT9 = np.ascontiguousarray(np.concatenate(
        [larger_w[int(eval_t)].T, larger_b[int(eval_t)].reshape(1, D)],
        axis=0)).astype(np.float32)
    ones_row = np.ones((1, 32 * B), dtype=np.float32)
    selT = np.tile(np.eye(CAP, dtype=np.float32), (NTASKS, 1))
    repT = np.tile(np.eye(CAP, dtype=np.float32), (1, A))
    ident = np.eye(128, dtype=np.float32)

    in_maps = []
    for c in range(N_CORES):
        xT_c = np.ascontiguousarray(
            x[c * BL:(c + 1) * BL].reshape(TOK, D).T).astype(np.float32)
        # reorder the contraction index to i2 = c2*L + l, then k-tile:
        # rw_c[p, (r, k, o)] = route_weights[c, r, l(k,p)*CAP + c2(k,p), o]
        rw2 = route_weights[c, :A].reshape(A, L, CAP, L).transpose(0, 2, 1, 3)
        rw_c = np.ascontiguousarray(
            rw2.reshape(A, IT, 128, L)
            .transpose(2, 0, 1, 3).reshape(128, A * IT * L)).astype(
                np.float32)
        in_maps.append({
            "xT": xT_c, "weffT": weffT, "beff_col": beff_col, "rw": rw_c,
            "wlT9": wlT9, "ones_row": ones_row, "selT": selT, "repT": repT,
            "ident": ident,
        })
    return A, in_maps


def kernel(**inputs):
    from concourse.bass_utils import run_bass_kernel_spmd

    A, in_maps = _host_prep(**inputs)
    if A not in _CACHE:
        _CACHE[A] = _build(A)
    nc = _CACHE[A]
    res = run_bass_kernel_spmd(nc, in_maps, core_ids=list(range(N_CORES)))
    return np.concatenate(
        [res.results[c]["out"] for c in range(N_CORES)], axis=0)
